# revision 21
# baseline (speedup 1.0000x reference)
"""Trainium2 Bass kernel for nn_BilinearInterpolation_60670708023631.

Math: the reference pads the (128,128,32) image into a (128,128,65,32) volume
that is zero everywhere except depth slab z=32, trilinearly samples it at
64*64*65 transformed grid points, and sums over the 65 depth samples per
output pixel.  Because the volume is a single slab, each sample reduces to a
2D 4-corner gather weighted by a z-slab weight zw = fz0*[z0==32]+fz1*[z1==32].
The 4 corners always live in the 2x2 patch at (y0, x0), so we gather one
512-byte patch-table row per sample and fold corner selection into 4 weights.

zw is nonzero only where the (affine in k) z coordinate crosses [31, 33) —
for a given transformation that is a contiguous window of at most
ceil(2/|dz/dk|)+1 of the 65 depth samples per pixel.  The kernel computes the
per-pixel window start on device and gathers/reduces only KW samples per
pixel; KW is chosen host-side from the transformation's z-slope (falling back
to wider windows or the fully dense variant when the slope is shallow), so
the result is exact for every input.

Sharding: 4096 output pixels split across 8 cores (512 each); the patch table
is replicated.
"""
import numpy as np

import concourse.bass as bass
import concourse.bacc as bacc
import concourse.mybir as mybir
import concourse.tile as tile
from concourse import bass_utils, library_config

P = 128          # partitions
KD = 65          # depth samples per pixel
NS = 4           # pixel slots per partition (512 pixels / 128)
C = 32           # channels
N_CORES = 8
OUT_H = OUT_W = 64
H = W = 128

f32 = mybir.dt.float32
i32 = mybir.dt.int32
i16 = mybir.dt.int16
OP = mybir.AluOpType
AF = mybir.ActivationFunctionType

_CACHE: dict = {}

# jnp.linspace(-1, 1, 64, dtype=float32), bit-exact (differs from np.linspace)
_XY_LIN_HEX = (
    "000080bf7edf77bffcbe6fbf7a9e67bff87d5fbf765d57bff43c4fbf721c47bf"
    "f0fb3ebf6edb36bfecba2ebf6a9a26bfe8791ebf655916bfe4380ebf611806bf"
    "bfeffbbeb9aeebbeb76ddbbeb12ccbbeafebbabea9aaaabea7699abea1288abe"
    "39cf73be314d53be29cb32be214912be318ee3bd218aa2bd210c43bd010882bc"
    "4008823c400c433d308aa23d418ee33d2849123e31cb323e394d533e41cf733e"
    "a4288a3ea9699a3eadaaaa3eb1ebba3eb52ccb3eb96ddb3ebdaeeb3ec1effb3e"
    "6418063fe6380e3f6859163fea791e3f6c9a263feeba2e3f70db363ff2fb3e3f"
    "741c473ff63c4f3f785d573ffa7d5f3f7c9e673ffebe6f3f80df773f0000803f"
)
XY_LIN = np.frombuffer(bytes.fromhex(_XY_LIN_HEX), dtype=np.float32)


def _fma32(a, b, c):
    """float32 fused multiply-add via exact float64 intermediate."""
    return np.float32(np.float64(a) * np.float64(b) + np.float64(c))


def _split12(a):
    """Dekker split of float32 into high/low halves (<=12 mantissa bits each)."""
    a = np.float32(a)
    c = np.float32(a * np.float32(2 ** 12 + 1))
    ah = np.float32(c - np.float32(c - a))
    return ah, np.float32(a - ah)


def _build_program(kw, debug_taps=False):
    """kw = depth-window size per pixel; kw == KD means dense (no windowing)."""
    dense = kw == KD
    F = NS * kw                  # gathered points per partition
    nc = bacc.Bacc("TRN2", target_bir_lowering=False, debug=False)

    tab = nc.dram_tensor("tab", (H * W, 4 * C), f32, kind="ExternalInput")
    trep = nc.dram_tensor("trep", (P, 11), f32, kind="ExternalInput")
    base2 = nc.dram_tensor("base2", (P, 3 * NS), f32, kind="ExternalInput")
    jr = nc.dram_tensor("jr", (P, F), f32, kind="ExternalInput")  # j-ramp / z-ramp
    scr = nc.dram_tensor("scr", (P, F), i16)  # DRAM bounce for index rewrap
    out_d = nc.dram_tensor("out", (NS, P, C), f32, kind="ExternalOutput")
    if debug_taps:
        dbg_idx = nc.dram_tensor("dbg_idx", (P, F), i16, kind="ExternalOutput")
        dbg_w = nc.dram_tensor("dbg_w", (P, F * 4), f32, kind="ExternalOutput")
        dbg_kst = nc.dram_tensor("dbg_kst", (P, NS), f32, kind="ExternalOutput")
        dbg_z = nc.dram_tensor("dbg_z", (P, F), f32, kind="ExternalOutput")

    with tile.TileContext(nc) as tc:
        with (
            tc.tile_pool(name="const", bufs=1) as cp,
            tc.tile_pool(name="work", bufs=1) as wp,
            tc.tile_pool(name="gath", bufs=2) as gp,
            tc.tile_pool(name="tmp", bufs=2) as tp,
            tc.tile_pool(name="outp", bufs=2) as op_,
        ):
            nc.gpsimd.load_library(library_config.mlp)

            # ---- load constants
            t_t = cp.tile([P, 11], f32)
            nc.sync.dma_start(out=t_t[:], in_=trep[:])
            b2_t = cp.tile([P, 3 * NS], f32)
            nc.scalar.dma_start(out=b2_t[:], in_=base2[:])
            jr_t = cp.tile([P, F], f32)
            nc.scalar.dma_start(out=jr_t[:], in_=jr[:])

            def tcol(j):
                return t_t[:, j:j + 1]

            # floor(v) for any v: r = rne_int(v); floor = r - (r > v)
            def floor_(x, name, shape):
                ri = wp.tile(shape, i32, tag=f"fl_ri{name}")
                nc.vector.tensor_copy(out=ri[:], in_=x[:])
                r = wp.tile(shape, f32, tag=f"fl_r{name}")
                nc.vector.tensor_copy(out=r[:], in_=ri[:])
                g_ = wp.tile(shape, f32, tag=f"fl_g{name}")
                nc.vector.tensor_tensor(out=g_[:], in0=r[:], in1=x[:], op=OP.is_gt)
                nc.vector.tensor_tensor(out=r[:], in0=r[:], in1=g_[:],
                                        op=OP.subtract)
                return r

            # trunc toward zero on a whole tile: sign(x) * floor(|x|)
            def trunc_(x, name, shape):
                a_ = wp.tile(shape, f32, tag=f"tr_a{name}")
                nc.scalar.activation(out=a_[:], in_=x[:], func=AF.Abs)
                fl = floor_(a_, f"t{name}", shape)
                sg = wp.tile(shape, f32, tag=f"tr_s{name}")
                nc.scalar.activation(out=sg[:], in_=x[:], func=AF.Sign)
                xt = wp.tile(shape, f32, tag=f"t{name}")
                nc.vector.tensor_tensor(out=xt[:], in0=fl[:], in1=sg[:],
                                        op=OP.mult)
                return xt

            # ceil(v) for any v: r = rne_int(v); ceil = r + (r < v)
            def ceil_(x, name, shape):
                ri = wp.tile(shape, i32, tag=f"cl_ri{name}")
                nc.vector.tensor_copy(out=ri[:], in_=x[:])
                r = wp.tile(shape, f32, tag=f"cl_r{name}")
                nc.vector.tensor_copy(out=r[:], in_=ri[:])
                g_ = wp.tile(shape, f32, tag=f"cl_g{name}")
                nc.vector.tensor_tensor(out=g_[:], in0=r[:], in1=x[:], op=OP.is_lt)
                nc.vector.tensor_tensor(out=r[:], in0=r[:], in1=g_[:], op=OP.add)
                return r

            # trep cols: [t2h,t2l,t3]*3 coords, then t2_z full (9), rcz (10)
            # base2 cols: A2[coord r, slot t] = fma(T[r,1], yg, fl(T[r,0]*xg))
            def a2col(r):
                return b2_t[:, r * NS:(r + 1) * NS]

            if not dense:
                # ---- window start per pixel slot (approximate math is fine:
                # it only positions the window; out-of-window samples it keeps
                # have zw == 0 exactly)
                # Z(k) ~ czk*k + Z0; Z0 ~ 32.5*(A2z - t2z + t3z + 1)
                z0 = wp.tile([P, NS], f32)
                nc.vector.tensor_scalar(out=z0[:], in0=a2col(2),
                                        scalar1=tcol(9), scalar2=tcol(8),
                                        op0=OP.subtract, op1=OP.add)
                nc.vector.tensor_scalar(out=z0[:], in0=z0[:], scalar1=1.0,
                                        scalar2=32.5, op0=OP.add, op1=OP.mult)
                # klo = min((31-Z0)*rcz, (33-Z0)*rcz); kst = clip(ceil(klo))
                a = wp.tile([P, NS], f32)
                nc.vector.tensor_scalar(out=a[:], in0=z0[:], scalar1=-1.0,
                                        scalar2=31.0, op0=OP.mult, op1=OP.add)
                nc.vector.tensor_scalar(out=a[:], in0=a[:],
                                        scalar1=tcol(10), scalar2=None,
                                        op0=OP.mult)
                b = wp.tile([P, NS], f32)
                nc.vector.tensor_scalar(out=b[:], in0=z0[:], scalar1=-1.0,
                                        scalar2=33.0, op0=OP.mult, op1=OP.add)
                nc.vector.tensor_scalar(out=b[:], in0=b[:],
                                        scalar1=tcol(10), scalar2=None,
                                        op0=OP.mult)
                nc.vector.tensor_tensor(out=a[:], in0=a[:], in1=b[:], op=OP.min)
                kc = ceil_(a, "k", [P, NS])
                kst = wp.tile([P, NS], f32)
                nc.vector.tensor_scalar(out=kst[:], in0=kc[:], scalar1=0.0,
                                        scalar2=float(KD - kw), op0=OP.max,
                                        op1=OP.min)
                if debug_taps:
                    nc.sync.dma_start(out=dbg_kst[:], in_=kst[:])

            # ---- coordinates, batched: CO = [X | Y | Z] as [P, 3F]
            # Bit-exact replication of the reference fp path:
            #   s = fma(t3, 1, fma(t2, zl, A2));  coord = scale2 * fl(s + 1)
            # zl has <=6-bit mantissas so t2*zl = Ph + Pl exactly with split t2;
            # the fma rounds once — recovered with a 2Sum-compensated add.
            if dense:
                zl = jr_t
            else:
                # zlin(kst + j) = (kst + j)/32 - 1  (exact in f32)
                u_ = wp.tile([P, F], f32)
                nc.vector.tensor_tensor(
                    out=u_[:].rearrange("p (t k) -> p t k", t=NS),
                    in0=jr_t[:].rearrange("p (t k) -> p t k", t=NS),
                    in1=kst[:].broadcast_to([P, NS, kw]),
                    op=OP.add)
                zl = wp.tile([P, F], f32)
                nc.vector.tensor_scalar(out=zl[:], in0=u_[:],
                                        scalar1=1.0 / 32.0, scalar2=-1.0,
                                        op0=OP.mult, op1=OP.add)
            CO = wp.tile([P, 3 * F], f32)
            for ci in range(3):
                o = 3 * ci
                A2b = a2col(ci).broadcast_to([P, NS, kw])
                def v3(t_):  # [P, F] view as (p, slot, k)
                    return t_[:].rearrange("p (t k) -> p t k", t=NS)
                Ph = wp.tile([P, F], f32, tag="cPh")
                nc.vector.tensor_scalar(out=Ph[:], in0=zl[:], scalar1=tcol(o),
                                        scalar2=None, op0=OP.mult)
                Pl = wp.tile([P, F], f32, tag="cPl")
                nc.vector.tensor_scalar(out=Pl[:], in0=zl[:], scalar1=tcol(o + 1),
                                        scalar2=None, op0=OP.mult)
                # Knuth 2Sum(A2, Ph) -> u, er
                u = wp.tile([P, F], f32, tag="cu")
                nc.vector.tensor_tensor(out=v3(u), in0=v3(Ph), in1=A2b, op=OP.add)
                bv = wp.tile([P, F], f32, tag="cbv")
                nc.vector.tensor_tensor(out=v3(bv), in0=v3(u), in1=A2b,
                                        op=OP.subtract)   # bv = u - A2 (~Ph)
                av = wp.tile([P, F], f32, tag="cav")
                nc.vector.tensor_tensor(out=av[:], in0=u[:], in1=bv[:],
                                        op=OP.subtract)   # av = u - bv (~A2)
                br = wp.tile([P, F], f32, tag="cbr")
                nc.vector.tensor_tensor(out=br[:], in0=Ph[:], in1=bv[:],
                                        op=OP.subtract)   # br = Ph - bv
                ar = wp.tile([P, F], f32, tag="car")
                nc.vector.tensor_tensor(out=v3(ar), in0=v3(av), in1=A2b,
                                        op=OP.subtract)   # ar = av - A2 = -(A2-av)
                er = wp.tile([P, F], f32, tag="cer")
                nc.vector.tensor_tensor(out=er[:], in0=br[:], in1=ar[:],
                                        op=OP.subtract)   # er = br - ar
                nc.vector.tensor_tensor(out=er[:], in0=er[:], in1=Pl[:],
                                        op=OP.add)        # q = er + Pl
                nc.vector.tensor_tensor(out=u[:], in0=u[:], in1=er[:],
                                        op=OP.add)        # A3 = u + q
                # v = fl(fl(A3 + t3) + 1); coord = scale2 * v (exact *64; *32.5
                # matches fl(65*(v/2)))
                co = CO[:, ci * F:(ci + 1) * F]
                nc.vector.tensor_scalar(out=u[:], in0=u[:], scalar1=tcol(o + 2),
                                        scalar2=1.0, op0=OP.add, op1=OP.add)
                nc.vector.tensor_scalar(out=co, in0=u[:],
                                        scalar1=(64.0 if ci < 2 else 32.5),
                                        scalar2=None, op0=OP.mult)
            if debug_taps:
                nc.sync.dma_start(out=dbg_z[:], in_=CO[:, 2 * F:3 * F])

            # ---- trunc + clip0 (batched over XYZ), then gather indices ASAP
            T3 = trunc_(CO, "all", [P, 3 * F])
            CF0 = wp.tile([P, 3 * F], f32)
            nc.vector.tensor_scalar(out=CF0[:], in0=T3[:], scalar1=0.0,
                                    scalar2=127.0, op0=OP.max, op1=OP.min)
            nc.vector.tensor_scalar(out=CF0[:, 2 * F:3 * F],
                                    in0=CF0[:, 2 * F:3 * F], scalar1=64.0,
                                    scalar2=None, op0=OP.min)

            # idx = Yf0*128 + Xf0 (int16)
            idxf = wp.tile([P, F], f32)
            nc.vector.tensor_scalar(out=idxf[:], in0=CF0[:, F:2 * F],
                                    scalar1=128.0, scalar2=None, op0=OP.mult)
            nc.vector.tensor_tensor(out=idxf[:], in0=idxf[:],
                                    in1=CF0[:, 0:F], op=OP.add)
            idxi = wp.tile([P, F], i16)
            nc.vector.tensor_copy(out=idxi[:], in_=idxf[:])
            if debug_taps:
                nc.sync.dma_start(out=dbg_idx[:], in_=idxi[:])

            # ---- rewrap indices into dma_gather's 16-partition wrapped layout:
            # wrapped[q + 16r, f*8 + w] = idxi[16w + q, f]
            nc.sync.dma_start(out=scr[:], in_=idxi[:])
            wT = wp.tile([P, F * 8], i16)
            for r in range(8):
                eng = nc.sync if r % 2 == 0 else nc.scalar
                eng.dma_start(
                    out=wT[16 * r:16 * r + 16, :].rearrange(
                        "q (w f) -> q w f", f=F),
                    in_=bass.AP(scr, 0, [[F, 16], [16 * F, 8], [1, F]]))
            wrp = wp.tile([P, F * 8], i16)
            nc.vector.tensor_copy(
                out=wrp[:].rearrange("p (f w) -> p w f", w=8),
                in_=wT[:].rearrange("p (w f) -> p w f", f=F))

            # ---- weights (overlap the gather descriptor generation)
            CF1 = wp.tile([P, 3 * F], f32)
            nc.vector.tensor_scalar(out=CF1[:], in0=T3[:], scalar1=1.0,
                                    scalar2=0.0, op0=OP.add, op1=OP.max)
            nc.vector.tensor_scalar(out=CF1[:], in0=CF1[:], scalar1=127.0,
                                    scalar2=None, op0=OP.min)
            nc.vector.tensor_scalar(out=CF1[:, 2 * F:3 * F],
                                    in0=CF1[:, 2 * F:3 * F], scalar1=64.0,
                                    scalar2=None, op0=OP.min)

            FB0 = wp.tile([P, 3 * F], f32)   # [fx0 | fy0 | fz0]
            nc.vector.tensor_tensor(out=FB0[:], in0=CF1[:], in1=CO[:],
                                    op=OP.subtract)
            FB1 = wp.tile([P, 3 * F], f32)   # [fx1 | fy1 | fz1]
            nc.vector.tensor_tensor(out=FB1[:], in0=CO[:], in1=CF0[:],
                                    op=OP.subtract)
            DXY = wp.tile([P, 2 * F], f32)   # [dx | dy]
            nc.vector.tensor_tensor(out=DXY[:], in0=CF1[:, 0:2 * F],
                                    in1=CF0[:, 0:2 * F], op=OP.subtract)

            fx0, fx1 = FB0[:, 0:F], FB1[:, 0:F]
            fy0, fy1 = FB0[:, F:2 * F], FB1[:, F:2 * F]
            fz0, fz1 = FB0[:, 2 * F:3 * F], FB1[:, 2 * F:3 * F]
            dx, dy = DXY[:, 0:F], DXY[:, F:2 * F]

            # zw = fz0*[Zf0==32] + fz1*[Zf1==32]
            e0 = wp.tile([P, F], f32, tag="e0")
            nc.vector.tensor_scalar(out=e0[:], in0=CF0[:, 2 * F:3 * F],
                                    scalar1=32.0, scalar2=None, op0=OP.is_equal)
            nc.vector.tensor_tensor(out=e0[:], in0=e0[:], in1=fz0, op=OP.mult)
            e1 = wp.tile([P, F], f32, tag="e1")
            nc.vector.tensor_scalar(out=e1[:], in0=CF1[:, 2 * F:3 * F],
                                    scalar1=32.0, scalar2=None, op0=OP.is_equal)
            nc.vector.tensor_tensor(out=e1[:], in0=e1[:], in1=fz1, op=OP.mult)
            zw = wp.tile([P, F], f32, tag="zw")
            nc.vector.tensor_tensor(out=zw[:], in0=e0[:], in1=e1[:], op=OP.add)

            # rf0 = (fx0 + (1-dy)*fx1)*zw ; rf1 = dy*fx1*zw
            # cf0 = fy0 + (1-dx)*fy1     ; cf1 = dx*fy1
            rf1 = wp.tile([P, F], f32, tag="rf1")
            nc.vector.tensor_tensor(out=rf1[:], in0=dy, in1=fx1, op=OP.mult)
            rf0 = wp.tile([P, F], f32, tag="rf0")
            nc.vector.tensor_tensor(out=rf0[:], in0=fx0, in1=fx1, op=OP.add)
            nc.vector.tensor_tensor(out=rf0[:], in0=rf0[:], in1=rf1[:],
                                    op=OP.subtract)
            nc.vector.tensor_tensor(out=rf0[:], in0=rf0[:], in1=zw[:], op=OP.mult)
            nc.vector.tensor_tensor(out=rf1[:], in0=rf1[:], in1=zw[:], op=OP.mult)

            cf1 = wp.tile([P, F], f32, tag="cf1")
            nc.vector.tensor_tensor(out=cf1[:], in0=dx, in1=fy1, op=OP.mult)
            cf0 = wp.tile([P, F], f32, tag="cf0")
            nc.vector.tensor_tensor(out=cf0[:], in0=fy0, in1=fy1, op=OP.add)
            nc.vector.tensor_tensor(out=cf0[:], in0=cf0[:], in1=cf1[:],
                                    op=OP.subtract)

            # W slots interleaved [f][s]
            wfull = wp.tile([P, F * 4], f32)
            for s, (a_, b_) in enumerate(((rf0, cf0), (rf0, cf1),
                                          (rf1, cf0), (rf1, cf1))):
                nc.vector.tensor_tensor(out=wfull[:, s::4], in0=a_[:], in1=b_[:],
                                        op=OP.mult)
            if debug_taps:
                nc.sync.dma_start(out=dbg_w[:], in_=wfull[:])

            # ---- gather + weighted reduce, chunked so DMA/desc-gen overlaps
            # the multiply/reduce of the previous chunk
            if dense:
                rounds = [(t * kw, kw) for t in range(NS)]
            else:
                rounds = [(0, 2 * kw), (2 * kw, kw), (3 * kw, kw)]
            for (f0, nf) in rounds:
                g = gp.tile([P, nf * 4 * C], f32, tag="g")
                nc.gpsimd.dma_gather(
                    out_ap=g[:].rearrange("p (k e) -> p k e", e=4 * C),
                    in_ap=tab[:],
                    idxs_ap=wrp[:, f0 * 8:(f0 + nf) * 8],
                    num_idxs=nf * P,
                    num_idxs_reg=nf * P,
                    elem_size=4 * C,
                    single_packet=False,
                )
                fr = nf * 4
                tmp = tp.tile([P, nf * 4 * C], f32, tag="tmp")
                nc.vector.tensor_tensor(
                    out=tmp[:].rearrange("p (c f) -> p f c", f=fr),
                    in0=g[:].rearrange("p (f c) -> p f c", c=C),
                    in1=wfull[:, f0 * 4:(f0 + nf) * 4].broadcast_to([P, fr, C]),
                    op=OP.mult)
                # per-slot reduce over this round's f-range
                tv = tmp[:].rearrange("p (c f) -> p c f", f=fr)
                for t in range(NS):
                    lo = t * kw * 4 - f0 * 4
                    if lo < 0 or lo >= fr:
                        continue
                    o = op_.tile([P, C], f32, tag="o")
                    nc.vector.tensor_reduce(
                        out=o[:], in_=tv[:, :, lo:lo + kw * 4],
                        axis=mybir.AxisListType.X, op=OP.add)
                    nc.sync.dma_start(out=out_d[t], in_=o[:])

    nc.compile()
    return nc


def _pick_kw(transformation):
    T = np.asarray(transformation, dtype=np.float32).reshape(3, 4)
    czk = abs(float(T[2, 2])) * 65.0 / 64.0   # |dz_voxel/dk|
    if czk == 0.0:
        return KD
    width = 2.0 / czk
    for kw in (6, 8, 12, 16, 24, 32, 48):
        if width <= kw - 1.5:
            return kw
    return KD


def _host_prep(image, transformation, kw):
    img = np.ascontiguousarray(np.asarray(image, dtype=np.float32)[0])  # (H, W, C)
    T = np.asarray(transformation, dtype=np.float32).reshape(3, 4)

    xp1 = np.minimum(np.arange(W) + 1, W - 1)
    yp1 = np.minimum(np.arange(H) + 1, H - 1)
    tab = np.concatenate(
        [img, img[:, xp1], img[yp1], img[yp1][:, xp1]], axis=2
    ).reshape(H * W, 4 * C)

    f = np.float32
    cols = []
    for r in range(3):
        t2h, t2l = _split12(T[r, 2])
        cols += [t2h, t2l, T[r, 3]]
    cols.append(T[2, 2])                      # col 9: t2_z
    czk = f(T[2, 2] * f(32.5) / f(32.0))
    cols.append(f(1.0) / czk if czk != 0 else f(0.0))  # col 10: rcz
    trep = np.tile(np.array(cols, dtype=f)[None, :], (P, 1))

    if kw == KD:
        jr = np.tile(((np.arange(KD) - 32) / 32).astype(f), (P, NS))
    else:
        jr = np.tile(np.arange(kw, dtype=f), (P, NS))

    in_maps = []
    for c in range(N_CORES):
        pix = c * 512 + np.arange(NS)[None, :] * P + np.arange(P)[:, None]  # (P, NS)
        xgp = XY_LIN[pix % OUT_W]
        ygp = XY_LIN[pix // OUT_W]
        b2 = np.empty((P, 3 * NS), dtype=f)
        for r in range(3):
            a1 = f(T[r, 0] * xgp)
            b2[:, r * NS:(r + 1) * NS] = _fma32(T[r, 1], ygp, a1)
        in_maps.append({
            "tab": tab,
            "trep": trep,
            "base2": b2,
            "jr": jr,
        })
    return in_maps


def _run(in_maps, kw, trace=False):
    nc = _CACHE.get(kw)
    if nc is None:
        nc = _build_program(kw)
        _CACHE[kw] = nc
    res = bass_utils.run_bass_kernel_spmd(
        nc, in_maps, core_ids=list(range(N_CORES)), trace=trace)
    out_full = np.empty((N_CORES * 512, C), dtype=np.float32)
    for c in range(N_CORES):
        out_full[c * 512:(c + 1) * 512] = res.results[c]["out"].reshape(512, C)
    return out_full.reshape(1, OUT_H, OUT_W, C), res


def kernel(image, transformation):
    kw = _pick_kw(transformation)
    in_maps = _host_prep(image, transformation, kw)
    out, _ = _run(in_maps, kw, trace=False)
    return out
